# revision 9
# baseline (speedup 1.0000x reference)
"""Trainium2 Bass kernel for nn_CBAE_EndToEnd (soft differentiable rasterizer).

Full inputs in, full outputs out. Shards the 192 frames across 8 NeuronCores
(24 frames/core, SPMD). Per-frame pipeline on each core (layout: primitives
on partitions, pixels on the free dim):

  - edge affine functions  arg = orient/SOFT * s = A*gx + B*gy + C  evaluated
    as bf16 3-way-split matmuls on TensorE (contract=15, exact products, fp32
    PSUM accumulate), row-tiled 4-concurrent (K=15 <= 32)
  - ACT sigmoid (the only table set used -> no table switches)
  - coverage product over the 12 edges: balanced multiply tree split across
    VectorE (6 ops) and GpSimdE (5 ops); final mul fused with the
    alpha*sigmoid(alive) scale via scalar_tensor_tensor
  - compositing: one_m = 1 - a (DVE), transpose 128x128 blocks (PE), forward
    cumprod via DVE tensor_tensor_scan (primitives pre-sorted by DESCENDING z
    on host so the reference's exclusive reverse cumprod becomes a forward
    scan), transpose back reading through a spacer column of ones to realize
    the exclusive shift, w = a * t_excl, fp32 color matmul, one DMA per frame.

Host side (numpy): depth sort, shoelace orientation, coefficient build,
bf16 splits, identity matrix.
"""

import numpy as np
import ml_dtypes

H = 128
W = 128
N = 128
K = 12
SOFT = 0.01
T_TOTAL = 192
N_CORES = 8
F = T_TOTAL // N_CORES  # frames per core

bf16 = ml_dtypes.bfloat16

_PAIRS = [(0, 0), (0, 1), (1, 0), (0, 2), (2, 0), (1, 1)]

_CACHE = {}


def _split3(x):
    x = np.asarray(x, np.float32)
    h = x.astype(bf16)
    r = x - h.astype(np.float32)
    m = r.astype(bf16)
    l = (r - m.astype(np.float32)).astype(bf16)
    return h, m, l


def _host_prep(trajectory, colors, alpha, z, csg):
    """Returns per-core input maps."""
    T = trajectory.shape[0]
    od = np.argsort(z, kind="stable")[::-1]  # descending z == forward compositing
    traj = np.asarray(trajectory, np.float32)[:, 0, :]
    P = traj[:, : N * K * 2].reshape(T, N, K, 2)[:, od]
    alive = traj[:, N * K * 2:][:, od]
    v0 = P
    v1 = np.roll(P, -1, axis=2)
    e = v1 - v0
    area2 = np.sum(v0[..., 0] * v1[..., 1] - v1[..., 0] * v0[..., 1], axis=2)
    orient = np.sign(area2).astype(np.float32)[:, :, None]
    A = (-orient * e[..., 1] / SOFT).astype(np.float32)  # [T,N,K] gx coef
    B = (orient * e[..., 0] / SOFT).astype(np.float32)   # gy coef
    C = (orient * (e[..., 1] * v0[..., 0] - e[..., 0] * v0[..., 1]) / SOFT).astype(
        np.float32)
    sig_alive = 1.0 / (1.0 + np.exp(-alive.astype(np.float32)))
    aeff = (np.asarray(alpha, np.float32)[od][None, :] * sig_alive).astype(
        np.float32)  # [T, N]
    ckeep = (
        np.asarray(colors, np.float32)[0][od]
        * (1.0 - np.asarray(csg)[od].astype(np.float32))[:, None]
    ).astype(np.float32)  # [N, 3]

    # --- static G15 [15, H*W] bf16 ---
    ys = ((np.arange(H) + 0.5) / H).astype(np.float32)
    xs = ((np.arange(W) + 0.5) / W).astype(np.float32)
    gx = np.tile(xs, H)
    gy = np.repeat(ys, W)
    gxp = _split3(gx)
    gyp = _split3(gy)
    ones = np.ones(H * W, np.float32)
    G15 = np.stack(
        [gxp[j] for (_, j) in _PAIRS]
        + [gyp[j] for (_, j) in _PAIRS]
        + [ones, ones, ones]
    ).astype(bf16)  # [15, HW]

    # --- per-frame lhsT W15 packed for row-tiling ---
    # tile jp == edge k; every tile's partition layout is n (all 128 prims).
    # physical packing: quad q = k//4, slot i = k%4
    # w15[t, 32*i + row, q*128 + n] = split piece for (n, k)
    Ap = _split3(A)
    Bp = _split3(B)
    Cp = _split3(C)
    w15 = np.zeros((T, 128, 384), np.float32)
    for k in range(12):
        q, i = k // 4, k % 4
        col = slice(q * 128, q * 128 + 128)
        for r, (ui, _) in enumerate(_PAIRS):
            w15[:, 32 * i + r, col] = Ap[ui][:, :, k].astype(np.float32)
            w15[:, 32 * i + 6 + r, col] = Bp[ui][:, :, k].astype(np.float32)
        for ui in range(3):
            w15[:, 32 * i + 12 + ui, col] = Cp[ui][:, :, k].astype(np.float32)
    w15 = w15.astype(bf16)

    ident = np.eye(128, dtype=np.float32)

    in_maps = []
    for c in range(N_CORES):
        fr = slice(c * F, (c + 1) * F)
        in_maps.append({
            "g15": np.ascontiguousarray(G15),
            "ident": ident,
            "ckeep": ckeep,
            "w15": np.ascontiguousarray(w15[fr]),
            "aeff": np.ascontiguousarray(aeff[fr].T),  # [128, F]
        })
    return in_maps


def _build_nc(n_frames):
    import concourse.bass as bass
    import concourse.bacc as bacc
    import concourse.tile as tile
    from concourse import mybir
    from contextlib import ExitStack

    dt = mybir.dt
    AF = mybir.ActivationFunctionType
    ALU = mybir.AluOpType

    nc = bacc.Bacc(None)
    g15_d = nc.dram_tensor("g15", [15, H * W], dt.bfloat16, kind="ExternalInput")
    ident_d = nc.dram_tensor("ident", [128, 128], dt.float32, kind="ExternalInput")
    ckeep_d = nc.dram_tensor("ckeep", [128, 3], dt.float32, kind="ExternalInput")
    w15_d = nc.dram_tensor(
        "w15", [n_frames, 128, 384], dt.bfloat16, kind="ExternalInput")
    aeff_d = nc.dram_tensor("aeff", [128, n_frames], dt.float32,
                            kind="ExternalInput")
    out_d = nc.dram_tensor("out", [n_frames, H, W, 3], dt.float32,
                           kind="ExternalOutput")

    NPIX = H * W          # 16384
    PT = 1024             # pixels per tile
    NT = NPIX // PT       # 16 tiles/frame

    # multiply-tree schedule: (engine, out_name, in0, in1)
    # DVE: 6 ops, GPSIMD: 5 ops (last DVE op fused with aeff scale)
    TREE = [
        ("v", "m0", "s0", "s1"),
        ("g", "m1", "s2", "s3"),
        ("v", "m2", "s4", "s5"),
        ("g", "m3", "s6", "s7"),
        ("v", "m4", "s8", "s9"),
        ("g", "m5", "s10", "s11"),
        ("v", "n0", "m0", "m1"),
        ("g", "n1", "m2", "m3"),
        ("v", "n2", "m4", "m5"),
        ("g", "p0", "n0", "n1"),
    ]

    with tile.TileContext(nc) as tc:
        with ExitStack() as ctx:
            singles = ctx.enter_context(tc.tile_pool(name="singles", bufs=1))
            w15_pool = ctx.enter_context(tc.tile_pool(name="w15", bufs=2))
            sig_pool = ctx.enter_context(tc.tile_pool(name="sig", bufs=14))
            tmp_pool = ctx.enter_context(tc.tile_pool(name="tmp", bufs=12))
            a_pool = ctx.enter_context(tc.tile_pool(name="a", bufs=2))
            om_pool = ctx.enter_context(tc.tile_pool(name="om", bufs=2))
            ti_pool = ctx.enter_context(tc.tile_pool(name="ti", bufs=2))
            w_pool = ctx.enter_context(tc.tile_pool(name="w", bufs=3))
            fb_pool = ctx.enter_context(tc.tile_pool(name="fb", bufs=2))
            s_psum = ctx.enter_context(
                tc.tile_pool(name="s_ps", bufs=3, space="PSUM"))
            t_psum = ctx.enter_context(
                tc.tile_pool(name="t_ps", bufs=1, space="PSUM"))
            c_psum = ctx.enter_context(
                tc.tile_pool(name="c_ps", bufs=1, space="PSUM"))

            # ---- static loads ----
            g15_sb = singles.tile([128, H * W], dt.bfloat16)
            for i in range(4):
                nc.sync.dma_start(out=g15_sb[32 * i:32 * i + 15, :], in_=g15_d[:])
            ident_sb = singles.tile([128, 128], dt.float32)
            nc.sync.dma_start(out=ident_sb, in_=ident_d[:])
            ckeep_sb = singles.tile([128, 3], dt.float32)
            nc.sync.dma_start(out=ckeep_sb, in_=ckeep_d[:])
            aeff_sb = singles.tile([128, n_frames], dt.float32)
            nc.sync.dma_start(out=aeff_sb, in_=aeff_d[:])

            for t in range(n_frames):
                w15_sb = w15_pool.tile([128, 384], dt.bfloat16, tag="w15")
                nc.sync.dma_start(out=w15_sb, in_=w15_d[t])

                fb_sb = fb_pool.tile([128, NT * 24], dt.float32, tag="fb")
                for pt in range(NT):
                    pt0 = pt * PT
                    vals = {}
                    for jp in range(12):
                        q, i = jp // 4, jp % 4
                        s_ps = s_psum.tile([128, PT], dt.float32, tag="s")
                        for c in range(2):
                            nc.tensor.matmul(
                                s_ps[:, c * 512:(c + 1) * 512],
                                lhsT=w15_sb[32 * i:32 * i + 15,
                                            q * 128:(q + 1) * 128],
                                rhs=g15_sb[32 * i:32 * i + 15,
                                           pt0 + c * 512:pt0 + (c + 1) * 512],
                                start=True, stop=True,
                                tile_position=(32 * i, 0),
                            )
                        sg = sig_pool.tile([128, PT], dt.float32, tag="sig")
                        nc.scalar.activation(sg, s_ps, AF.Sigmoid)
                        vals[f"s{jp}"] = sg

                    for eng, dst, a_, b_ in TREE:
                        o = tmp_pool.tile([128, PT], dt.float32, tag="tmp")
                        engine = nc.vector if eng == "v" else nc.gpsimd
                        engine.tensor_mul(o, vals[a_], vals[b_])
                        vals[dst] = o
                    a_sb = a_pool.tile([128, PT], dt.float32, tag="a")
                    nc.vector.scalar_tensor_tensor(
                        out=a_sb, in0=vals["n2"], scalar=aeff_sb[:, t:t + 1],
                        in1=vals["p0"], op0=ALU.mult, op1=ALU.mult)

                    om_sb = om_pool.tile([128, PT], dt.float32, tag="om")
                    nc.vector.tensor_scalar(
                        om_sb, a_sb, -1.0, 1.0, ALU.mult, ALU.add)
                    # ti layout per 128-block: [1.0 spacer | scan result(128)]
                    ti_sb = ti_pool.tile([128, 8 * 129], dt.float32, tag="ti")
                    ti_strided = bass.AP(
                        tensor=ti_sb.tensor, offset=ti_sb.offset,
                        ap=[ti_sb.ap[0], [129, 8], [1, 1]])
                    nc.vector.memset(ti_strided, 1.0)
                    co_ps = c_psum.tile([128, 24], dt.float32, tag="co")
                    for hh in range(2):
                        t_ps = t_psum.tile([128, 512], dt.float32, tag="tp")
                        for b in range(4):
                            blk = hh * 4 + b
                            nc.tensor.transpose(
                                t_ps[:, b * 128:(b + 1) * 128],
                                om_sb[:, blk * 128:(blk + 1) * 128],
                                ident_sb)
                        for b in range(4):
                            blk = hh * 4 + b
                            nc.vector.tensor_tensor_scan(
                                out=ti_sb[:, blk * 129 + 1:blk * 129 + 129],
                                data0=t_ps[:, b * 128:(b + 1) * 128],
                                data1=om_sb[:, blk * 128:(blk + 1) * 128],
                                initial=1.0, op0=ALU.mult, op1=ALU.bypass)
                        tb_ps = t_psum.tile([128, 512], dt.float32, tag="tp")
                        for b in range(4):
                            blk = hh * 4 + b
                            nc.tensor.transpose(
                                tb_ps[:, b * 128:(b + 1) * 128],
                                ti_sb[:, blk * 129:blk * 129 + 128],
                                ident_sb)
                        w_sb = w_pool.tile([128, 512], dt.float32, tag="w")
                        nc.vector.tensor_mul(
                            w_sb, a_sb[:, hh * 512:(hh + 1) * 512], tb_ps)
                        for b in range(4):
                            blk = hh * 4 + b
                            nc.tensor.matmul(
                                co_ps[:, blk * 3:(blk + 1) * 3],
                                lhsT=w_sb[:, b * 128:(b + 1) * 128],
                                rhs=ckeep_sb,
                                start=True, stop=True)
                    nc.vector.tensor_copy(fb_sb[:, pt * 24:(pt + 1) * 24], co_ps)
                # frame output DMA: fb[c, (tile, r_l, ch)] -> out[t, r, c, ch]
                src = fb_sb.rearrange("c (tl rl ch) -> c tl rl ch", rl=8, ch=3)
                dst = out_d[t].rearrange("(tl rl) c ch -> c tl rl ch", rl=8)
                nc.sync.dma_start(out=dst, in_=src)
    nc.finalize()
    return nc


def _get_program(n_frames):
    if n_frames not in _CACHE:
        _CACHE[n_frames] = _build_nc(n_frames)
    return _CACHE[n_frames]


def _enable_jax_cache():
    try:
        import jax
        if jax.config.jax_compilation_cache_dir is None:
            jax.config.update("jax_compilation_cache_dir", "/tmp/jax_bass_cache")
            jax.config.update("jax_persistent_cache_min_entry_size_bytes", -1)
            jax.config.update("jax_persistent_cache_min_compile_time_secs", 0.5)
    except Exception:
        pass


def kernel(trajectory, colors, alpha, z, csg):
    from concourse.bass_utils import run_bass_kernel_spmd

    _enable_jax_cache()

    in_maps = _host_prep(
        np.asarray(trajectory), np.asarray(colors), np.asarray(alpha),
        np.asarray(z), np.asarray(csg))
    nc = _get_program(F)
    res = run_bass_kernel_spmd(nc, in_maps, core_ids=list(range(N_CORES)))
    outs = [res.results[c]["out"] for c in range(N_CORES)]
    video = np.concatenate(outs, axis=0)  # [192, H, W, 3]
    return video[None].astype(np.float32)  # [1, 192, H, W, 3]


if __name__ == "__main__":
    nc = _build_nc(2)
    print("built ok")
